# revision 22
# baseline (speedup 1.0000x reference)
"""Bass/Trainium2 kernel for nn_BiGAT (2-layer GAT, scatter-softmax message passing).

Strategy (dst-sharded, 8 cores, v4):
  Host: append self-loops, sort edges by (dst-block, src-half, src), give
  each core a contiguous dst range (6250 nodes).  Within a core, edges are
  grouped into 128-dst "blocks"; each block's edge list is padded to a
  uniform tile count (t_lo lo-half + t_hi hi-half tiles of 128 edges) so
  one SPMD program fits all cores.  Pad edges point at sentinel table rows
  whose att-src value is -1e30, so exp() kills their softmax weight.

  Per-edge rows are fetched with dma_gather (the batched SWDGE gather):
  ~1us of Pool time per CALL but only ~0.34ns per row, so each block's
  whole gather is 2 calls (int16 indices force a lo/hi table split at row
  32768).  Table rows are 768B ([h1(256)|es(8)|ed(8)|pad] bf16) to satisfy
  the 256B-multiple stride requirement; node v lives at row v+1 and rows 0
  / n+1 are the lo/hi sentinels.  ed for a block's 128 dst nodes is a
  single 128-descriptor indirect DMA via element_offset.

  The scatter one-hots S[e,d] and their transposes ST[d,e] (used to expand
  ed to edges via PE) are precomputed on the host and streamed as fp8
  (exact for 0/1), removing the DVE is_equal builds entirely; fp8 lhsT x
  bf16 rhs matmul is verified exact on HW.  These streams and the index
  streams are shared by both layers.

  K1 (per core): phase A computes [h1|es|ed] = x @ [W1|W1@As|W1@Ad] from a
  host-pretransposed bf16 x, writing h1tab.  Phase B per block: gather,
  e = es + ST^T@ed_blk, p = exp(leaky_relu(e)) as max(exp(e), exp(0.2e)),
  p*h1 via one 4D broadcast multiply per 4-tile group, scatter-matmul
  S^T @ [p*h1|p] accumulated in PSUM.  Epilogue: divide by denominator,
  b1 + ELU, then the layer-2 node record [h2_pre|es2|ed2].  Host:
  all-gather records into the (padded, 256B-row) layer-2 table.  K2: same
  machinery, 1 head / 16 channels -> output slices; host adds b2.
"""
import sys

sys.path.insert(0, "/opt/trn_rl_repo")

import numpy as np
import ml_dtypes
import concourse.bass as bass
import concourse.bacc as bacc
import concourse.tile as tile
from concourse import mybir
from concourse.bass_utils import run_bass_kernel_spmd
from concourse.masks import make_identity

F32 = mybir.dt.float32
F32R = mybir.dt.float32r
I32 = mybir.dt.int32
I16 = mybir.dt.int16
U8 = mybir.dt.uint8
BF16 = mybir.dt.bfloat16
FP8 = mybir.dt.float8e4

# problem dims (hardcoded per contract)
N, IN, HID, HEADS, NCLS = 50000, 128, 32, 8, 16
HC = HEADS * HID            # 256
ROW = HC + 2 * HEADS        # 272 = used cols of a K1 table row [h1|es|ed]
RW1 = 384                   # K1 table row stride (768B, 256B-multiple)
RW2 = 128                   # K2 table row stride (256B): [h2|es2|ed2|pad]
MM = HC + HEADS             # 264 = K1 scatter-matmul rhs [p*h1|p]
NEG = 0.2                   # leaky_relu slope
NCORES = 8
P = 128
HIB = 32768                 # int16 index limit -> lo/hi table split row
NEG_BIG = -1e30
EPS = 1e-30
REC = 18                    # layer-2 node record: h2_pre(16) | es2 | ed2
KT = 4                      # tiles per compute group


GMAX = 8   # dma_gather HW limit: at most 1024 (=8*128) indices per call
NQ = 4     # SWDGE queues (ucode max); queue-parallel desc-gen is ~2.2x
# CoreSim enforces a strict semaphore<->SWDGE-queue lock that Tile's
# round-robin sem assignment cannot satisfy with rotating queues; HW runs
# (incl. exact-match gather tests) show multi-queue is fine.  Sim runs build
# single-queue (identical dataflow) via this flag.
SINGLE_QUEUE = False


def _call_plan(t_lo, t_hi):
    """Fast path: exactly 3 gathers per block (2 lo + 1 hi) + the ed
    indirect = 4 Pool DMAs, so Tile's 8 round-robin SWDGE semaphores land on
    (queue, sem) pairs that are consistent across blocks with the queue map
    [0,1,2,3].  Returns (fast, [(c0, k, queue), ...])."""
    fast = 4 <= t_lo <= 4 * GMAX and 3 <= t_hi <= 3 * GMAX
    if not fast:
        plan = []
        for sec0, cnt in ((0, t_lo), (t_lo, t_hi)):
            for c0 in range(0, cnt, GMAX):
                plan.append((sec0 + c0, min(GMAX, cnt - c0), 0))
        return False, plan
    qs = (0,) * 7 if SINGLE_QUEUE else (1, 2, 3, 0, 1, 2, 3)
    plan = []
    c0 = 0
    for i, a in enumerate(np.array_split(np.arange(t_lo), 4)):
        plan.append((c0, len(a), qs[i]))
        c0 += len(a)
    for i, a in enumerate(np.array_split(np.arange(t_hi), 3)):
        plan.append((c0, len(a), qs[4 + i]))
        c0 += len(a)
    return True, plan


def _emit_block_pool_dmas(nc, ed_fn, rhs, ixs_t, tab_ap, n, hibase,
                          t_lo, t_hi, rw, regs):
    """Emit a block's Pool-side DMAs: the ed indirect (always SWDGE queue
    0) plus the lo/hi dma_gathers per _call_plan.  regs carries per-call
    runtime index counts (None -> compile-time full count; trailing pad
    indices are then -1 and generate no descriptors/traffic)."""
    fast, plan = _call_plan(t_lo, t_hi)
    if SINGLE_QUEUE:
        # sim-only: quiet the uninitialized-SBUF canary on slots skipped by
        # runtime-count gathers (on HW those slots hold the previous
        # occupant's finite values and are zeroed out of the result by the
        # all-zero one-hot columns of pad slots)
        nc.vector.memset(rhs[:], 0.0)
    ed_fn()
    for i, (c0, k, qn) in enumerate(plan):
        a, b = (0, min(HIB, n + 2)) if c0 < t_lo else (hibase, n + 2)
        nc.gpsimd.dma_gather(
            out_ap=rhs[:, c0 * rw:(c0 + k) * rw]
                .rearrange("p (g e) -> p g e", e=rw),
            in_ap=tab_ap[a:b],
            idxs_ap=ixs_t[:, c0 * 8:(c0 + k) * 8], num_idxs=k * P,
            num_idxs_reg=(regs[i] if regs is not None else k * P),
            elem_size=rw, queue_num=qn)


# ----------------------------------------------------------------- host prep
def _prep_edges(src, dst, n, ncores):
    """Sort by (dst-block, src-half, src), shard by dst range, pad each
    block's lo/hi edge sections to uniform tile counts (t_lo, t_hi).
    Node v -> table row v+1; rows 0 and n+1 are sentinels.  Streams:
      ixs  i16 [nb, P, (t_lo+t_hi)*8]: dma_gather indices, 16-wrapped and
                                       replicated across partition stripes
      grw  i32 [nb, P, 1]:             dst table row per partition
      soh  fp8 [nb, P, (2*tpb)*P]:     S one-hots | ST one-hots
    """
    npc = n // ncores
    nb = (npc + P - 1) // P
    hibase = HIB if n + 2 > HIB else (n + 2) // 2
    percore = []
    for c in range(ncores):
        m = (dst >= c * npc) & (dst < (c + 1) * npc)
        s, dl = src[m], dst[m] - c * npc
        hi = (s + 1 >= hibase).astype(np.int64)
        blk = dl // P
        order = np.lexsort((s, hi, blk))
        s, dl, hi = s[order], dl[order], hi[order]
        nlo = np.bincount(blk[order], weights=1 - hi, minlength=nb).astype(int)
        nall = np.bincount(blk[order], minlength=nb)
        percore.append((s, dl, nlo, nall))
    t_lo = max(1, max(int(np.ceil((nl).max() / P))
                      for _, _, nl, na in percore))
    t_hi = max(1, max(int(np.ceil((na - nl).max() / P))
                      for _, _, nl, na in percore))
    tpb = t_lo + t_hi
    sent_hi = (n + 1 - hibase)
    fast, plan = _call_plan(t_lo, t_hi)
    streams = []
    darange = np.arange(P, dtype=np.int64)
    for c in range(ncores):
        s, dl, nlo, nall = percore[c]
        # slot tables: idx (table row in section coords) and dloc per slot
        idx = np.empty((nb, tpb, P), np.int64)
        idx[:, :t_lo] = 0
        idx[:, t_lo:] = sent_hi
        dloc = np.full((nb, tpb, P), 255, np.int64)
        cnts = np.zeros((nb, 8), np.int32)
        off = 0
        for b in range(nb):
            klo = int(nlo[b])
            kall = int(nall[b])
            khi = kall - klo
            es, ed = s[off:off + kall], dl[off:off + kall]
            if False:
                # runtime-count blocks disabled: full sentinel-padded gathers
                idx[b, :t_lo] = -1
                idx[b, t_lo:] = -1
            lo_flat = idx[b, :t_lo].reshape(-1)
            lo_flat[:klo] = es[:klo] + 1
            hi_flat = idx[b, t_lo:].reshape(-1)
            hi_flat[:khi] = es[klo:] + 1 - hibase
            dl_lo = dloc[b, :t_lo].reshape(-1)
            dl_lo[:klo] = ed[:klo] - b * P
            dl_hi = dloc[b, t_lo:].reshape(-1)
            dl_hi[:khi] = ed[klo:] - b * P
            for i, (c0, k, _q) in enumerate(plan):
                sec_real = klo if c0 < t_lo else khi
                rel = sec_real - (c0 - (0 if c0 < t_lo else t_lo)) * P
                cnt_i = max(0, min(k * P, rel))
                if True:
                    cnt_i = k * P
                elif cnt_i == 0:
                    # ensure >=1 valid index per call (leading sentinel)
                    flat = idx[b].reshape(-1)
                    flat[c0 * P] = 0 if c0 < t_lo else sent_hi
                    cnt_i = 1
                cnts[b, i] = cnt_i
            off += kall
        # 16-wrap the per-section index lists, replicate across stripes
        ixs = np.empty((nb, P, tpb * 8), np.int16)
        lo = idx[:, :t_lo].reshape(nb, t_lo * P)
        hi = idx[:, t_lo:].reshape(nb, t_hi * P)
        for arr, c0, tt in ((lo, 0, t_lo), (hi, t_lo * 8, t_hi)):
            w = arr.reshape(nb, tt * 8, 16)          # [nb, col, lane]
            ixs[:, :, c0:c0 + tt * 8] = \
                w.transpose(0, 2, 1)[:, darange % 16, :].astype(np.int16)
        # one-hots: S[e, t*P+d] and ST[d, (tpb+t)*P+e]
        soh = np.zeros((nb, P, 2 * tpb * P), ml_dtypes.float8_e4m3)
        sS = (dloc.transpose(0, 2, 1)[:, :, :, None] == darange)  # [nb,e,t,d]
        soh[:, :, 0:tpb * P] = sS.reshape(nb, P, tpb * P)
        sT = (dloc[:, None, :, :] == darange[None, :, None, None])  # [nb,d,t,e]
        soh[:, :, tpb * P:] = sT.reshape(nb, P, tpb * P)
        # dst table rows (pads -> sentinel row 0)
        bb = np.arange(nb)[:, None]
        grow = c * npc + bb * P + darange[None, :]
        grw = np.where(grow >= (c + 1) * npc, 0, grow + 1)[:, :, None]
        streams.append({
            "ixs": np.ascontiguousarray(ixs),
            "grw": np.ascontiguousarray(grw.astype(np.int32)),
            "soh": np.ascontiguousarray(soh),
            "cnt": np.ascontiguousarray(cnts),
        })
    return streams, t_lo, t_hi, nb, npc, hibase


# ------------------------------------------------------------------ K1 build
def _build_k1(n, npc, nb, t_lo, t_hi, hibase):
    tpb = t_lo + t_hi
    nc = bacc.Bacc("TRN2", target_bir_lowering=False, debug=False,
                   num_swdge_queues=1 if SINGLE_QUEUE else NQ)
    ncols = ((n + 511) // 512) * 512
    xT_d = nc.dram_tensor("xT", [IN, ncols], BF16, kind="ExternalInput")
    w1e_d = nc.dram_tensor("w1ext", [IN, ROW], BF16, kind="ExternalInput")
    w2p_d = nc.dram_tensor("w2pack", [P, 2 * NCLS], F32R, kind="ExternalInput")
    a2p_d = nc.dram_tensor("a2pack", [NCLS, 2], F32R, kind="ExternalInput")
    b1b_d = nc.dram_tensor("b1bc", [P, HC], F32, kind="ExternalInput")
    ixs_d = nc.dram_tensor("ixs", [nb, P, tpb * 8], I16, kind="ExternalInput")
    grw_d = nc.dram_tensor("grw", [nb, P, 1], I32, kind="ExternalInput")
    soh_d = nc.dram_tensor("soh", [nb, P, 2 * tpb * P], FP8,
                           kind="ExternalInput")
    rec_d = nc.dram_tensor("h2rec", [npc, REC], F32, kind="ExternalOutput")
    h1tab = nc.dram_tensor("h1tab", [n + 2, RW1], BF16, kind="Internal")

    ng = (n + 511) // 512
    with tile.TileContext(nc) as tc:
        with (
            tc.tile_pool(name="consts", bufs=1) as cp,
            tc.tile_pool(name="sba", bufs=3) as sba,
            tc.tile_pool(name="psa", bufs=4, space="PSUM") as psa,
        ):
            w1e_t = cp.tile([IN, ROW], BF16)
            nc.sync.dma_start(out=w1e_t[:], in_=w1e_d.ap()[:])

            # ---- phase A: h1tab rows [h1|es|ed] at node+1
            for g in range(ng):
                c0 = g * 512
                rows_g = min(512, n - c0)
                xT_t = sba.tile([IN, 512], BF16, tag="xT")
                nc.sync.dma_start(out=xT_t[:], in_=xT_d.ap()[:, c0:c0 + 512])
                h_big = sba.tile([P, 4 * ROW], BF16, tag="h_big")
                nj = (rows_g + P - 1) // P
                for j in range(nj):
                    rows_j = min(P, rows_g - j * P)
                    h_ps = psa.tile([P, ROW], F32, tag="h_ps")
                    nc.tensor.matmul(out=h_ps[:rows_j],
                                     lhsT=xT_t[:, j * P:j * P + rows_j],
                                     rhs=w1e_t[:], start=True, stop=True)
                    if j % 2 == 0:
                        nc.scalar.copy(out=h_big[:rows_j, j * ROW:(j + 1) * ROW],
                                       in_=h_ps[:rows_j, :])
                    else:
                        nc.vector.tensor_copy(
                            out=h_big[:rows_j, j * ROW:(j + 1) * ROW],
                            in_=h_ps[:rows_j, :])
                if rows_g == 512:
                    nc.sync.dma_start(
                        out=h1tab.ap()[c0 + 1:c0 + 513].rearrange(
                            "(j p) r -> p j r", p=P)[:, :, 0:ROW],
                        in_=h_big[:].rearrange("p (j r) -> p j r", r=ROW))
                else:
                    for j in range(nj):
                        rows_j = min(P, rows_g - j * P)
                        r0 = c0 + 1 + j * P
                        nc.sync.dma_start(
                            out=h1tab.ap()[r0:r0 + rows_j, 0:ROW],
                            in_=h_big[:rows_j, j * ROW:(j + 1) * ROW])
            # sentinel rows 0 and n+1: h1=0, es=-1e30, ed=0
            sent_t = cp.tile([1, ROW], BF16)
            nc.vector.memset(sent_t[:1, 0:HC], 0.0)
            nc.vector.memset(sent_t[:1, HC:HC + HEADS], NEG_BIG)
            nc.vector.memset(sent_t[:1, HC + HEADS:ROW], 0.0)
            nc.sync.dma_start(out=h1tab.ap()[0:1, 0:ROW], in_=sent_t[:1, :])
            nc.sync.dma_start(out=h1tab.ap()[n + 1:n + 2, 0:ROW],
                              in_=sent_t[:1, :])

        tc.strict_bb_all_engine_barrier()

        # ---- phase B: blocks of 128 dst nodes
        with (
            tc.tile_pool(name="bconsts", bufs=1) as bc,
            tc.tile_pool(name="sbb", bufs=6) as sbb,
            tc.tile_pool(name="rhp", bufs=5) as rhp,
            tc.tile_pool(name="ssb", bufs=3) as ssb,
            tc.tile_pool(name="mmp", bufs=4) as mmp,
            tc.tile_pool(name="accp", bufs=2, space="PSUM") as accp,
            tc.tile_pool(name="eps", bufs=2, space="PSUM") as eps_p,
            tc.tile_pool(name="xpp", bufs=1, space="PSUM") as xpp,
            tc.tile_pool(name="smp", bufs=1, space="PSUM") as smp,
        ):
            ident2 = bc.tile([P, P], F32)
            make_identity(nc, ident2[:])
            b1b_t = bc.tile([P, HC], F32)
            nc.sync.dma_start(out=b1b_t[:], in_=b1b_d.ap()[:])
            w2_t = bc.tile([P, 2 * NCLS], F32R)
            nc.sync.dma_start(out=w2_t[:], in_=w2p_d.ap()[:])
            a2_t = bc.tile([NCLS, 2], F32R)
            nc.sync.dma_start(out=a2_t[:], in_=a2p_d.ap()[:])
            for b in range(nb):
                nrows = min(P, npc - b * P)
                ixs_t = sbb.tile([P, tpb * 8], I16, tag="ixs")
                nc.sync.dma_start(out=ixs_t[:], in_=ixs_d.ap()[b])
                grw_t = sbb.tile([P, 1], I32, tag="grw")
                nc.sync.dma_start(out=grw_t[:], in_=grw_d.ap()[b])
                soh_t = sbb.tile([P, 2 * tpb * P], FP8, tag="soh")
                nc.sync.dma_start(out=soh_t[:], in_=soh_d.ap()[b])
                ed_blk = sbb.tile([P, HEADS], BF16, tag="edblk")

                def _ed():
                    nc.gpsimd.indirect_dma_start(
                        out=ed_blk[:], out_offset=None, in_=h1tab.ap()[:],
                        in_offset=bass.IndirectOffsetOnAxis(
                            ap=grw_t[:, 0:1], axis=0),
                        element_offset=HC + HEADS)
                rhs = rhp.tile([P, tpb * RW1], BF16, tag="rhs")
                _emit_block_pool_dmas(nc, _ed, rhs, ixs_t, h1tab.ap(), n,
                                      hibase, t_lo, t_hi, RW1, None)

                acc = accp.tile([P, MM], F32, tag="acc")
                for t0 in range(0, tpb, KT):
                    k = min(KT, tpb - t0)
                    e_ps = eps_p.tile([P, KT * HEADS], F32, tag="eps")
                    for i in range(k):
                        nc.tensor.matmul(
                            out=e_ps[:, i * HEADS:(i + 1) * HEADS],
                            lhsT=soh_t[:, (tpb + t0 + i) * P:
                                       (tpb + t0 + i + 1) * P],
                            rhs=ed_blk[:], start=True, stop=True)
                    r3 = rhs[:, t0 * RW1:(t0 + k) * RW1] \
                        .rearrange("p (t r) -> p t r", r=RW1)
                    e_sb = ssb.tile([P, KT * HEADS], F32, tag="esb")
                    e3 = e_sb[:, 0:k * HEADS].rearrange("p (t r) -> p t r",
                                                        r=HEADS)
                    nc.vector.tensor_tensor(
                        out=e3, in0=r3[:, :, HC:HC + HEADS],
                        in1=e_ps[:, 0:k * HEADS]
                            .rearrange("p (t r) -> p t r", r=HEADS),
                        op=mybir.AluOpType.add)
                    # p = exp(leaky_relu(e)) = max(exp(e), exp(0.2e))
                    a_sb = ssb.tile([P, KT * HEADS], F32, tag="asb")
                    nc.scalar.activation(out=a_sb[:, 0:k * HEADS],
                                         in_=e_sb[:, 0:k * HEADS],
                                         func=mybir.ActivationFunctionType.Exp)
                    b_sb = ssb.tile([P, KT * HEADS], F32, tag="bsb")
                    nc.scalar.activation(out=b_sb[:, 0:k * HEADS],
                                         in_=e_sb[:, 0:k * HEADS],
                                         func=mybir.ActivationFunctionType.Exp,
                                         scale=NEG)
                    mm_t = mmp.tile([P, KT * MM], BF16, tag="mm")
                    m3 = mm_t[:, 0:k * MM].rearrange("p (t r) -> p t r", r=MM)
                    nc.vector.tensor_tensor(out=m3[:, :, HC:MM],
                                            in0=a_sb[:, 0:k * HEADS]
                                            .rearrange("p (t h) -> p t h",
                                                       h=HEADS),
                                            in1=b_sb[:, 0:k * HEADS]
                                            .rearrange("p (t h) -> p t h",
                                                       h=HEADS),
                                            op=mybir.AluOpType.max)
                    nc.vector.tensor_tensor(
                        out=m3[:, :, 0:HC].rearrange("p t (h c) -> p t h c",
                                                     c=HID),
                        in0=r3[:, :, 0:HC].rearrange("p t (h c) -> p t h c",
                                                     c=HID),
                        in1=m3[:, :, HC:MM].rearrange("p t (h c) -> p t h c",
                                                      c=1)
                            .to_broadcast([P, k, HEADS, HID]),
                        op=mybir.AluOpType.mult)
                    for i in range(k):
                        nc.tensor.matmul(out=acc[:],
                                         lhsT=soh_t[:, (t0 + i) * P:
                                                    (t0 + i + 1) * P],
                                         rhs=mm_t[:, i * MM:(i + 1) * MM],
                                         start=(t0 + i == 0),
                                         stop=(t0 + i == tpb - 1))

                # ---- block epilogue
                rd = ssb.tile([P, HEADS], F32, tag="rd")
                nc.vector.tensor_scalar_add(out=rd[:], in0=acc[:, HC:MM],
                                            scalar1=EPS)
                nc.vector.reciprocal(out=rd[:], in_=rd[:])
                hag = ssb.tile([P, HC], F32, tag="hag")
                a3 = acc[:, 0:HC].rearrange("p (h c) -> p h c", c=HID)
                rr3 = rd[:].rearrange("p (h c) -> p h c", c=1) \
                    .to_broadcast([P, HEADS, HID])
                nc.vector.tensor_tensor(
                    out=hag[:].rearrange("p (h c) -> p h c", c=HID),
                    in0=a3, in1=rr3, op=mybir.AluOpType.mult)
                nc.vector.tensor_add(out=hag[:], in0=hag[:], in1=b1b_t[:])
                # ELU: relu(x) + exp(min(x,0)) - 1
                rl = ssb.tile([P, HC], F32, tag="rl")
                nc.scalar.activation(out=rl[:], in_=hag[:],
                                     func=mybir.ActivationFunctionType.Relu)
                nc.vector.tensor_tensor(out=hag[:], in0=hag[:], in1=rl[:],
                                        op=mybir.AluOpType.subtract)
                nc.scalar.activation(out=hag[:], in_=hag[:],
                                     func=mybir.ActivationFunctionType.Exp)
                nc.vector.tensor_add(out=hag[:], in0=hag[:], in1=rl[:])
                nc.vector.tensor_scalar_add(out=hag[:], in0=hag[:], scalar1=-1.0)
                # h2_pre^T = W2^T @ h1^T ; es2/ed2 = a2^T @ h2_pre^T
                h2T_ps = smp.tile([NCLS, P], F32, tag="h2T")
                for half in range(2):
                    xp_ps = xpp.tile([P, P], F32, tag="xp")
                    nc.tensor.transpose(out=xp_ps[:],
                                        in_=hag[:, half * P:(half + 1) * P],
                                        identity=ident2[:])
                    h1T = ssb.tile([P, P], F32R, tag="h1T")
                    nc.vector.tensor_copy(out=h1T[:], in_=xp_ps[:])
                    nc.tensor.matmul(
                        out=h2T_ps[:],
                        lhsT=w2_t[:, half * NCLS:(half + 1) * NCLS],
                        rhs=h1T[:], start=(half == 0), stop=(half == 1))
                h2T_sb = ssb.tile([NCLS, P], F32R, tag="h2Tsb")
                nc.vector.tensor_copy(out=h2T_sb[:], in_=h2T_ps[:])
                ee_ps = smp.tile([2, P], F32, tag="ee")
                nc.tensor.matmul(out=ee_ps[:], lhsT=a2_t[:],
                                 rhs=h2T_sb[:], start=True, stop=True)
                ee_sb = ssb.tile([2, P], F32, tag="eesb")
                nc.vector.tensor_copy(out=ee_sb[:], in_=ee_ps[:])
                # transpose back to node-major, assemble the 18-col record
                recT_ps = smp.tile([P, REC], F32, tag="recT")
                nc.tensor.transpose(out=recT_ps[:, 0:NCLS],
                                    in_=h2T_sb[:].bitcast(F32),
                                    identity=ident2[:NCLS, :NCLS])
                nc.tensor.transpose(out=recT_ps[:, NCLS:REC], in_=ee_sb[:],
                                    identity=ident2[:2, :2])
                rec_sb = ssb.tile([P, REC], F32, tag="recsb")
                nc.vector.tensor_copy(out=rec_sb[:], in_=recT_ps[:])
                nc.sync.dma_start(out=rec_d.ap()[b * P:b * P + nrows],
                                  in_=rec_sb[:nrows])
    nc.compile()
    return nc


# ------------------------------------------------------------------ K2 build
def _build_k2(n, npc, nb, t_lo, t_hi, hibase):
    tpb = t_lo + t_hi
    nc = bacc.Bacc("TRN2", target_bir_lowering=False, debug=False,
                   num_swdge_queues=1 if SINGLE_QUEUE else NQ)
    tab_d = nc.dram_tensor("h2tab", [n + 2, RW2], BF16, kind="ExternalInput")
    ixs_d = nc.dram_tensor("ixs", [nb, P, tpb * 8], I16, kind="ExternalInput")
    grw_d = nc.dram_tensor("grw", [nb, P, 1], I32, kind="ExternalInput")
    soh_d = nc.dram_tensor("soh", [nb, P, 2 * tpb * P], FP8,
                           kind="ExternalInput")
    out_d = nc.dram_tensor("out2", [npc, NCLS], F32, kind="ExternalOutput")
    W = NCLS + 1  # 17 = scatter-matmul rhs [p*h2|p]

    with tile.TileContext(nc) as tc:
        with (
            tc.tile_pool(name="consts", bufs=1) as cp,
            tc.tile_pool(name="sbb", bufs=6) as sbb,
            tc.tile_pool(name="rhp", bufs=6) as rhp,
            tc.tile_pool(name="ssb", bufs=3) as ssb,
            tc.tile_pool(name="mmp", bufs=4) as mmp,
            tc.tile_pool(name="accp", bufs=2, space="PSUM") as accp,
            tc.tile_pool(name="eps", bufs=2, space="PSUM") as eps_p,
        ):
            for b in range(nb):
                nrows = min(P, npc - b * P)
                ixs_t = sbb.tile([P, tpb * 8], I16, tag="ixs")
                nc.sync.dma_start(out=ixs_t[:], in_=ixs_d.ap()[b])
                grw_t = sbb.tile([P, 1], I32, tag="grw")
                nc.sync.dma_start(out=grw_t[:], in_=grw_d.ap()[b])
                soh_t = sbb.tile([P, 2 * tpb * P], FP8, tag="soh")
                nc.sync.dma_start(out=soh_t[:], in_=soh_d.ap()[b])
                # [es2|ed2] of the block's 128 dst nodes (col 1 = ed2 used)
                edd = sbb.tile([P, 2], BF16, tag="edd")

                def _ed():
                    nc.gpsimd.indirect_dma_start(
                        out=edd[:], out_offset=None, in_=tab_d.ap()[:],
                        in_offset=bass.IndirectOffsetOnAxis(
                            ap=grw_t[:, 0:1], axis=0),
                        element_offset=NCLS)
                rhs = rhp.tile([P, tpb * RW2], BF16, tag="rhs")
                _emit_block_pool_dmas(nc, _ed, rhs, ixs_t, tab_d.ap(), n,
                                      hibase, t_lo, t_hi, RW2, None)

                acc = accp.tile([P, W], F32, tag="acc")
                for t0 in range(0, tpb, KT):
                    k = min(KT, tpb - t0)
                    e_ps = eps_p.tile([P, KT * 2], F32, tag="eps")
                    for i in range(k):
                        nc.tensor.matmul(
                            out=e_ps[:, 2 * i:2 * i + 2],
                            lhsT=soh_t[:, (tpb + t0 + i) * P:
                                       (tpb + t0 + i + 1) * P],
                            rhs=edd[:], start=True, stop=True)
                    r3 = rhs[:, t0 * RW2:(t0 + k) * RW2] \
                        .rearrange("p (t r) -> p t r", r=RW2)
                    e_sb = ssb.tile([P, KT], F32, tag="esb")
                    e3 = e_sb[:, 0:k].rearrange("p (t r) -> p t r", r=1)
                    nc.vector.tensor_tensor(
                        out=e3, in0=r3[:, :, NCLS:NCLS + 1],
                        in1=e_ps[:, 0:2 * k]
                            .rearrange("p (t r) -> p t r", r=2)[:, :, 1:2],
                        op=mybir.AluOpType.add)
                    a_sb = ssb.tile([P, KT], F32, tag="asb")
                    nc.scalar.activation(out=a_sb[:, 0:k], in_=e_sb[:, 0:k],
                                         func=mybir.ActivationFunctionType.Exp)
                    b_sb = ssb.tile([P, KT], F32, tag="bsb")
                    nc.scalar.activation(out=b_sb[:, 0:k], in_=e_sb[:, 0:k],
                                         func=mybir.ActivationFunctionType.Exp,
                                         scale=NEG)
                    mm_t = mmp.tile([P, KT * W], BF16, tag="mm")
                    m3 = mm_t[:, 0:k * W].rearrange("p (t r) -> p t r", r=W)
                    nc.vector.tensor_tensor(
                        out=m3[:, :, NCLS:W],
                        in0=a_sb[:, 0:k].rearrange("p (t h) -> p t h", h=1),
                        in1=b_sb[:, 0:k].rearrange("p (t h) -> p t h", h=1),
                        op=mybir.AluOpType.max)
                    nc.vector.tensor_tensor(
                        out=m3[:, :, 0:NCLS], in0=r3[:, :, 0:NCLS],
                        in1=m3[:, :, NCLS:W].to_broadcast([P, k, NCLS]),
                        op=mybir.AluOpType.mult)
                    for i in range(k):
                        nc.tensor.matmul(out=acc[:],
                                         lhsT=soh_t[:, (t0 + i) * P:
                                                    (t0 + i + 1) * P],
                                         rhs=mm_t[:, i * W:(i + 1) * W],
                                         start=(t0 + i == 0),
                                         stop=(t0 + i == tpb - 1))
                rd = ssb.tile([P, 1], F32, tag="rd")
                nc.vector.tensor_scalar_add(out=rd[:], in0=acc[:, NCLS:W],
                                            scalar1=EPS)
                nc.vector.reciprocal(out=rd[:], in_=rd[:])
                o_t = ssb.tile([P, NCLS], F32, tag="o")
                nc.vector.tensor_tensor(out=o_t[:], in0=acc[:, 0:NCLS],
                                        in1=rd[:].to_broadcast([P, NCLS]),
                                        op=mybir.AluOpType.mult)
                nc.sync.dma_start(out=out_d.ap()[b * P:b * P + nrows],
                                  in_=o_t[:nrows])
    nc.compile()
    return nc


# ------------------------------------------------------------------- driver
_CACHE = {}


def _get_programs(n, npc, nb, t_lo, t_hi, hibase, ncores):
    key = (n, npc, nb, t_lo, t_hi, hibase, ncores)
    if key not in _CACHE:
        _CACHE[key] = (_build_k1(n, npc, nb, t_lo, t_hi, hibase),
                       _build_k2(n, npc, nb, t_lo, t_hi, hibase))
    return _CACHE[key]


def kernel(x, edge_index, W1, att_src1, att_dst1, b1, W2, att_src2, att_dst2,
           b2, _ncores=NCORES, _trace=False):
    x = np.asarray(x, np.float32)
    edge_index = np.asarray(edge_index, np.int32)
    W1 = np.asarray(W1, np.float32)
    n = x.shape[0]
    loops = np.arange(n, dtype=np.int32)
    src = np.concatenate([edge_index[0], loops])
    dst = np.concatenate([edge_index[1], loops])
    streams, t_lo, t_hi, nb, npc, hibase = _prep_edges(src, dst, n, _ncores)

    # host-side packing
    ncols = ((n + 511) // 512) * 512
    xT = np.zeros((IN, ncols), ml_dtypes.bfloat16)
    xT[:, :n] = x.T.astype(ml_dtypes.bfloat16)
    A1s = np.zeros((HC, HEADS), np.float32)
    A1d = np.zeros((HC, HEADS), np.float32)
    for h in range(HEADS):
        A1s[h * HID:(h + 1) * HID, h] = np.asarray(att_src1, np.float32)[h]
        A1d[h * HID:(h + 1) * HID, h] = np.asarray(att_dst1, np.float32)[h]
    w1ext = np.concatenate([W1, W1 @ A1s, W1 @ A1d], axis=1) \
        .astype(ml_dtypes.bfloat16)                                # [128, 272]
    W2 = np.asarray(W2, np.float32)
    w2pack = np.concatenate([W2[0:P], W2[P:2 * P]], axis=1)        # [128, 32]
    a2pack = np.stack([np.asarray(att_src2, np.float32)[0],
                       np.asarray(att_dst2, np.float32)[0]], axis=1)  # [16, 2]
    b1bc = np.broadcast_to(np.asarray(b1, np.float32), (P, HC)).copy()

    k1, k2 = _get_programs(n, npc, nb, t_lo, t_hi, hibase, _ncores)

    in_maps1 = [{
        "xT": xT, "w1ext": w1ext, "w2pack": w2pack, "a2pack": a2pack,
        "b1bc": b1bc, "ixs": streams[c]["ixs"], "grw": streams[c]["grw"],
        "soh": streams[c]["soh"],
    } for c in range(_ncores)]
    res1 = run_bass_kernel_spmd(k1, in_maps1, core_ids=list(range(_ncores)),
                                trace=_trace)
    h2full = np.concatenate([res1.results[c]["h2rec"] for c in range(_ncores)])
    h2tab = np.zeros((n + 2, RW2), np.float32)
    h2tab[1:n + 1, 0:REC] = h2full
    h2tab[0, NCLS] = NEG_BIG       # lo sentinel es2 (ed2 stays 0)
    h2tab[n + 1, NCLS] = NEG_BIG   # hi sentinel
    h2tab = h2tab.astype(ml_dtypes.bfloat16)

    in_maps2 = [{
        "h2tab": h2tab, "ixs": streams[c]["ixs"], "grw": streams[c]["grw"],
        "soh": streams[c]["soh"],
    } for c in range(_ncores)]
    res2 = run_bass_kernel_spmd(k2, in_maps2, core_ids=list(range(_ncores)),
                                trace=_trace)
    out = np.concatenate([res2.results[c]["out2"] for c in range(_ncores)])
    out = out + np.asarray(b2, np.float32)[None, :]
    kernel._last = (res1, res2)
    return out
